# revision 35
# baseline (speedup 1.0000x reference)
"""Trainium2 Bass kernel for the MHA-with-diagonal-softmax module.

Computation (per batch b):
    q = rope(x @ Wq.T), k = rope(x @ Wk.T), v = x @ Wv.T      (per head, DH=128)
    sumexp[s,h] = sum_k exp(q_h[s] . k_h[k] * DH^-0.5)
    w = exp(q_h[s] . k_h[s] * DH^-0.5) / sumexp
    out = (w * v) @ Wo.T

Sharding: 8 cores = 2 (batch) x 4 (head groups of 4 heads).

Key structure (per core):
 - The negated score diagonal is precomputed per head (q*k mul + ones
   matmul) and fed to the exp as the ACT instruction's free per-row
   bias: exp(s_k - diag).  Then w = 1/rowsum(exp) directly - no
   diagonal extraction, no extra normalizer math.
 - Head 0's whole scores+exp+rowsum stream runs inside phase B, where
   ACT/DVE idle under the PE-bound Q/K projections.
 - Phase C streams seq-block-major over heads 1-3 with a one-iteration
   lag between the exp stream and the w/attn/output-projection stream,
   so the small-op chain never paces the engines.  The output
   projection accumulates all 4 heads and y DMA flows continuously.
Host sums the 4 per-core partials per batch.

On-chip dtype is fp16 with fp32 PSUM accumulation.
"""

import numpy as np
from contextlib import ExitStack

# Problem constants (hardcoded per harness contract).
B, S, D, H, DH = 2, 2048, 2048, 16, 128
HPC = 4            # heads per core
NHL = HPC * DH     # 512 local head dims per core
KB = D // 128      # 16 contraction blocks
SB = S // 128      # 16 seq blocks of 128
SC = S // 512      # 4 seq/emb chunks of 512
NCORES = 8

_CACHE = {}


def _build_nc():
    import concourse.bass as bass
    import concourse.tile as tile
    from concourse import bacc, mybir
    from concourse.masks import make_identity

    F16 = mybir.dt.float16
    F32 = mybir.dt.float32
    AF = mybir.ActivationFunctionType
    ALU = mybir.AluOpType
    AX = mybir.AxisListType

    nc = bacc.Bacc("TRN2", target_bir_lowering=False, debug=False)

    xT = nc.dram_tensor("xT", [D, S], F16, kind="ExternalInput").ap()
    wq = nc.dram_tensor("wq", [D, NHL], F16, kind="ExternalInput").ap()
    wk = nc.dram_tensor("wk", [D, NHL], F16, kind="ExternalInput").ap()
    wv = nc.dram_tensor("wv", [D, NHL], F16, kind="ExternalInput").ap()
    wo = nc.dram_tensor("wo", [NHL, D], F16, kind="ExternalInput").ap()
    ropeA = nc.dram_tensor("ropeA", [128, S], F16, kind="ExternalInput").ap()
    ropeB = nc.dram_tensor("ropeB", [128, S], F16, kind="ExternalInput").ap()
    y = nc.dram_tensor("y", [S, D], F16, kind="ExternalOutput").ap()

    xT_r = xT.rearrange("(a p) s -> p a s", p=128)
    wq_r = wq.rearrange("(a p) m -> p a m", p=128)
    wk_r = wk.rearrange("(a p) m -> p a m", p=128)
    wv_r = wv.rearrange("(a p) m -> p a m", p=128)
    wo_r = wo.rearrange("(h p) n -> p h n", p=128)

    with tile.TileContext(nc) as tc, ExitStack() as ctx:
        pool = ctx.enter_context(tc.tile_pool(name="sb", bufs=1))
        pp = ctx.enter_context(tc.tile_pool(name="ps", bufs=1, space="PSUM"))

        # ---- constants ----
        ra = pool.tile([128, S], F16, name="ra")
        rb = pool.tile([128, S], F16, name="rb")
        ident32 = pool.tile([128, 128], F32, name="ident32")
        make_identity(nc, ident32[:, :])
        ones1 = pool.tile([128, 128], F16, name="ones1")
        nc.gpsimd.memset(ones1[:, :], 1.0)
        onesn = pool.tile([128, 128], F16, name="onesn")
        nc.gpsimd.memset(onesn[:, :], -1.0)

        # ---- input DMAs (emission order == queue service order) ----
        xsb = pool.tile([128, KB, S], F16, name="xsb")

        def load_w(src_r, nblk, tag="w"):
            # bufs=1: wv/wo DMAs overwrite wk/wq's slots; the WAR dep on
            # the last wk/wq read orders them, and arrival still beats
            # first use by a wide margin.  One strided 3D DMA per weight:
            # per-block descriptors at 610ns each would serialize ~70us
            # of input loads on the sync queue.
            t = pool.tile([128, nblk, 512 * (KB // nblk)], F16, name="wt",
                          tag=tag, bufs=1)
            nc.sync.dma_start(t[:, :, :], src_r)
            return t

        wkt = pool.tile([128, KB, 512], F16, name="wkt", tag="w", bufs=1)
        for k0, k1 in ((0, 4), (4, 8), (8, 12), (12, 16)):
            nc.sync.dma_start(wkt[:, k0:k1, :], wk_r[:, k0:k1, :])
            nc.sync.dma_start(xsb[:, k0:k1, 0:512], xT_r[:, k0:k1, 0:512])
        nc.sync.dma_start(xsb[:, :, 512:1024], xT_r[:, :, 512:1024])
        wqt = load_w(wq_r, KB, tag="w2")
        for sc in range(2, SC):                      # rest of x, sc-major
            nc.sync.dma_start(xsb[:, :, sc * 512:(sc + 1) * 512],
                              xT_r[:, :, sc * 512:(sc + 1) * 512])
        # rope tables loaded after the startup-critical wk/x transfers
        # (SWDGE: a wide HWDGE DMA would fan out over several queues and
        # blow the DVE consumer's sync-wait budget)
        nc.gpsimd.dma_start(ra[:, :], ropeA[:, :])
        nc.gpsimd.dma_start(rb[:, :], ropeB[:, :])

        # ---- persistent per-head tiles ([head_dim, seq] layout) ----
        qh = [pool.tile([128, S], F16, name=f"qh{h}") for h in range(HPC)]
        kh = [pool.tile([128, S], F16, name=f"kh{h}") for h in range(HPC)]
        # v stored seq-block-major interleaved: [:, sq, h, :] so the attn
        # multiply covers all 4 heads of a block in one DVE op
        vAll = pool.tile([128, SB, HPC, 128], F16, name="vAll")

        # per-(sq,half,h) partial rowsums of exp(s-d); negated diag bias
        sumf2 = pool.tile([128, SB, 2, HPC], F32, name="sumf2")
        diagT = [pool.tile([128, SB], F32, name=f"diagT{h}")
                 for h in range(HPC)]

        def proj_chunk(wt, dst, mt, sc, on_dve=False):
            # dst <- (wt[:, :, mt] block).T @ x chunk
            ps = pp.tile([128, 512], F32, name="mmps", tag="mm", bufs=2)
            for kb in range(KB):
                nc.tensor.matmul(
                    ps[:, :],
                    wt[:, kb, mt * 128:(mt + 1) * 128],
                    xsb[:, kb, sc * 512:(sc + 1) * 512],
                    start=(kb == 0), stop=(kb == KB - 1))
            if on_dve:
                nc.vector.tensor_copy(dst, ps[:, :])
            else:
                nc.scalar.activation(dst, ps[:, :], AF.Copy)

        def rope(dst):
            # dst (in place): top = te*cos - to*sin ; bottom = te*sin + to*cos
            # ra = [cosT; cosT], rb = [-sinT; sinT]; swap = halves exchanged.
            for c in range(2):
                sl = slice(c * 1024, (c + 1) * 1024)
                swp = pool.tile([128, 1024], F16, name="swp", tag="swp", bufs=1)
                nc.gpsimd.dma_start(swp[0:64, :], dst[64:128, sl])
                nc.gpsimd.dma_start(swp[64:128, :], dst[0:64, sl])
                u = pool.tile([128, 1024], F16, name="u", tag="sc", bufs=2)
                nc.vector.tensor_mul(u[:, :], dst[:, sl], ra[:, sl])
                v2 = pool.tile([128, 1024], F16, name="v2", tag="sc", bufs=2)
                nc.vector.tensor_mul(v2[:, :], swp[:, :], rb[:, sl])
                nc.vector.tensor_add(dst[:, sl], u[:, :], v2[:, :])

        def diag_prep(h):
            # diagT[h][:, sq] = -(q.k) per position, as per-block columns
            # (queries on partitions) for the exp bias.
            ndrow = pool.tile([1, S], F32, name="ndrow", tag="ndrow", bufs=2)
            for c in range(2):
                sl = slice(c * 1024, (c + 1) * 1024)
                pr = pool.tile([128, 1024], F16, name="pr", tag="pr", bufs=2)
                nc.vector.tensor_mul(pr[:, :], qh[h][:, sl], kh[h][:, sl])
                for cc in range(2):
                    dps = pp.tile([128, 512], F32, name="dps", tag="mm",
                                  bufs=2)
                    nc.tensor.matmul(dps[:, :], onesn[:, :],
                                     pr[:, cc * 512:(cc + 1) * 512],
                                     start=True, stop=True)
                    o = (2 * c + cc) * 512
                    nc.vector.tensor_copy(ndrow[0:1, o:o + 512], dps[0:1, :])
            for sq in range(SB):   # partition-scatter row -> columns
                nc.gpsimd.dma_start(diagT[h][:, sq:sq + 1],
                                    ndrow[0:1, sq * 128:(sq + 1) * 128])

        def scores_head(h, sq, sum_on_act):
            # scores for 128 queries x all keys -> exp(s - diag) in two
            # halves (psum double-buffered); only the rowsums survive.
            # sum_on_act: per-half engine choice for the rowsum.
            if isinstance(sum_on_act, bool):
                sum_on_act = (sum_on_act, sum_on_act)
            ex_t = pool.tile([128, S], F16, name="ex", tag="ex", bufs=2)
            for half in range(2):
                sps = pp.tile([128, 1024], F32, name="sps", tag="sco", bufs=2)
                for c in range(2):
                    o = half * 1024 + c * 512
                    nc.tensor.matmul(sps[:, c * 512:(c + 1) * 512],
                                     qh[h][:, sq * 128:(sq + 1) * 128],
                                     kh[h][:, o:o + 512],
                                     start=True, stop=True)
                exd = ex_t[:, half * 1024:(half + 1) * 1024]
                if sum_on_act[half]:
                    nc.scalar.activation(
                        exd, sps[:, :], AF.Exp,
                        bias=diagT[h][:, sq:sq + 1],
                        accum_out=sumf2[:, sq, half, h:h + 1])
                else:
                    nc.scalar.activation(exd, sps[:, :], AF.Exp,
                                         bias=diagT[h][:, sq:sq + 1])
                    nc.vector.tensor_reduce(sumf2[:, sq, half, h:h + 1],
                                            exd, AX.X, ALU.add)

        def w_chain(sq):
            # w[q] = 1/sumexp per head -> [1,512] row -> broadcast -> attn
            ssum = pool.tile([128, HPC], F32, name="ssum", tag="ssum", bufs=2)
            nc.vector.tensor_add(ssum[:, :], sumf2[:, sq, 0, :],
                                 sumf2[:, sq, 1, :])
            rec = pool.tile([128, HPC], F32, name="rec", tag="rec", bufs=2)
            nc.vector.reciprocal(rec[:, :], ssum[:, :])
            tps = pp.tile([HPC, 128], F32, name="tps", tag="tps", bufs=1)
            nc.tensor.transpose(tps[:, :], rec[:, :], ident32[:, :])
            w4 = pool.tile([HPC, 128], F16, name="w4", tag="w4", bufs=2)
            nc.vector.tensor_copy(w4[:, :], tps[:, :])
            wrow = pool.tile([1, NHL], F16, name="wrow", tag="wrow", bufs=2)
            nc.gpsimd.dma_start(wrow[0:1, :], w4[:, :])
            bps = pp.tile([128, NHL], F32, name="bps", tag="mm", bufs=2)
            nc.tensor.matmul(bps[:, :], ones1[0:1, :], wrow[0:1, :],
                             start=True, stop=True)
            # one in-place mul scales all 4 heads' v for this block
            nc.vector.tensor_mul(vAll[:, sq, :, :], bps[:, :],
                                 vAll[:, sq, :, :])

        def oproj_piece(sq, ncx, on_act=False):
            # y[sq block, ncx chunk] = sum_h attnV[h].T @ wo rows
            ps = pp.tile([128, 512], F32, name="ops", tag="mm", bufs=2)
            for h in range(HPC):
                nc.tensor.matmul(
                    ps[:, :], vAll[:, sq, h, :],
                    wot[:, h, ncx * 512:(ncx + 1) * 512],
                    start=(h == 0), stop=(h == HPC - 1))
            yt = pool.tile([128, 512], F16, name="yt", tag="yt", bufs=2)
            if on_act:       # tail: ACT is idle once the exps are done
                nc.scalar.activation(yt[:, :], ps[:, :], AF.Copy)
            else:
                nc.vector.tensor_copy(yt[:, :], ps[:, :])
            nc.sync.dma_start(
                y[sq * 128:(sq + 1) * 128, ncx * 512:(ncx + 1) * 512],
                yt[:, :])

        # ================= phase B =====================================
        # K projection: sc-major while x streams in, then head-major so
        # each head's rope (DVE) starts ~30us earlier and the kh tiles
        # are all roped well before the head-0 score stream needs them
        kps = [pp.tile([128, 1024], F32, name=f"kps{i}", tag="sco", bufs=2)
               for i in range(2)]
        for kb in range(KB):
            for mt in range(HPC):
                nc.tensor.matmul(
                    kps[mt // 2][:, (mt % 2) * 512:(mt % 2 + 1) * 512],
                    wkt[:, kb, mt * 128:(mt + 1) * 128],
                    xsb[:, kb, 0:512],
                    start=(kb == 0), stop=(kb == KB - 1))
        for mt in range(HPC):
            nc.scalar.activation(kh[mt][:, 0:512],
                                 kps[mt // 2][:, (mt % 2) * 512:
                                              (mt % 2 + 1) * 512], AF.Copy)
        for mt in range(HPC):
            proj_chunk(wkt, kh[mt][:, 512:1024], mt, 1)
        for mt in range(HPC):
            for sc in (2, 3):
                proj_chunk(wkt, kh[mt][:, sc * 512:(sc + 1) * 512], mt, sc)
            rope(kh[mt])
        # wv reuses wk's slot, wo reuses wq's slot (tag bufs=2)
        wvt = load_w(wv_r, KB)
        wot = load_w(wo_r, HPC, tag="w2")
        vpre = [lambda h=h: proj_chunk(wvt, vAll[:, 0:4, h, :], h, 0,
                                       on_dve=True)
                for h in range(HPC)]

        # Q projection head-major: head 0 finishes first so its whole
        # scores/exp/rowsum stream runs here, under the PE-bound
        # projections (ACT and DVE are otherwise idle in phase B).
        h0q = []    # deferred head-0 score emissions
        for mt in range(HPC):
            for sc in range(SC):
                proj_chunk(wqt, qh[mt][:, sc * 512:(sc + 1) * 512], mt, sc)
                # two head-0 blocks per Q chunk, starting two chunks into
                # head 1: early enough that the ACT exp stream drains
                # before phase C, late enough that the first blocks don't
                # stall the PE queue on the rope/diag-bias chain
                if mt > 1 or sc >= 2:
                    # 3 per chunk: all 16 blocks emitted by the end of
                    # head 2, so the ACT exp backlog drains under head
                    # 3's projections instead of stalling phase C's start
                    for _ in range(3):
                        if h0q:
                            h0q.pop(0)()
                if mt >= 2 and len(vpre) > 1:
                    vpre.pop(0)()
            rope(qh[mt])
            diag_prep(mt)
            if mt == 0:
                h0q = [lambda sq=sq: scores_head(0, sq,
                                                 sum_on_act=bool(sq % 2))
                       for sq in range(SB)]
        # leftover first-quarter V chunks + head-0 blocks
        while vpre:
            vpre.pop(0)()
        while h0q:
            h0q.pop(0)()

        # ================= phase C: streaming over heads 1-3 ===========
        vfill = [(h, sc) for sc in range(1, SC) for h in range(HPC)]
        vstate = {"ps": None, "kb": 0, "h": 0, "sc": 0}

        def v_quarter():
            # four contraction steps of a V chunk; a full chunk is too
            # coarse to interleave between score bursts
            if vstate["ps"] is None:
                if not vfill:
                    return
                vstate["h"], vstate["sc"] = vfill.pop(0)
                vstate["kb"] = 0
                vstate["ps"] = pp.tile([128, 512], F32, name="vps",
                                       tag="vps", bufs=1)
            h, sc, ps = vstate["h"], vstate["sc"], vstate["ps"]
            k0 = vstate["kb"]
            for kb in range(k0, k0 + 4):
                nc.tensor.matmul(
                    ps[:, :], wvt[:, kb, h * 128:(h + 1) * 128],
                    xsb[:, kb, sc * 512:(sc + 1) * 512],
                    start=(kb == 0), stop=(kb == KB - 1))
            vstate["kb"] += 4
            if vstate["kb"] == KB:
                nc.vector.tensor_copy(vAll[:, 4 * sc:4 * sc + 4, h, :],
                                      ps[:, :])
                vstate["ps"] = None

        # rowsums: 3 of 6 half-blocks on ACT accum, 3 on DVE reduce -
        # balances the two engines
        for sq in range(SB):
            for h in (1, 2, 3):
                scores_head(h, sq, sum_on_act=(h == 1))
                if h == 1:
                    if sq > 0:
                        w_chain(sq - 1)
                    v_quarter()
                elif h == 2:
                    if sq > 1:
                        oproj_piece(sq - 2, 0)
                        oproj_piece(sq - 2, 1)
                    if sq == SB - 1:           # tail: pull in block 14
                        oproj_piece(sq - 1, 0, on_act=True)
                        oproj_piece(sq - 1, 1, on_act=True)
                    v_quarter()
                else:
                    if sq > 1:
                        oproj_piece(sq - 2, 2)
                        oproj_piece(sq - 2, 3)
                    if sq == SB - 1:
                        oproj_piece(sq - 1, 2, on_act=True)
                        oproj_piece(sq - 1, 3, on_act=True)
                    v_quarter()
                    v_quarter()
        w_chain(SB - 1)
        for ncx in range(4):
            oproj_piece(SB - 1, ncx, on_act=(ncx % 2 == 0))

    nc.compile()
    return nc


def _get_nc():
    if "nc" not in _CACHE:
        _CACHE["nc"] = _build_nc()
    return _CACHE["nc"]


_PERM = np.concatenate([np.arange(0, DH, 2), np.arange(1, DH, 2)])


def _host_inputs(x, rope_cos, rope_sin, Wq, Wk, Wv, Wo):
    """Build the 8 per-core input maps."""
    f16 = np.float16
    cosT = np.ascontiguousarray(np.asarray(rope_cos, np.float32)[0, :, 0, :].T)
    sinT = np.ascontiguousarray(np.asarray(rope_sin, np.float32)[0, :, 0, :].T)
    ra = np.concatenate([cosT, cosT], 0).astype(f16)
    rb = np.concatenate([-sinT, sinT], 0).astype(f16)

    Wq = np.asarray(Wq, np.float32)
    Wk = np.asarray(Wk, np.float32)
    Wv = np.asarray(Wv, np.float32)
    Wo = np.asarray(Wo, np.float32)
    x = np.asarray(x, np.float32)

    xTb = [np.ascontiguousarray(x[b].T).astype(f16) for b in range(B)]
    scale = DH ** -0.5

    in_maps = []
    for core in range(NCORES):
        b, g = divmod(core, HPC)
        hs = g * HPC
        rows = np.concatenate(
            [h * DH + _PERM for h in range(hs, hs + HPC)])      # deinterleave
        rows_v = np.arange(hs * DH, (hs + HPC) * DH)
        in_maps.append({
            "xT": xTb[b],
            "wq": np.ascontiguousarray((Wq[rows] * scale).T).astype(f16),
            "wk": np.ascontiguousarray(Wk[rows].T).astype(f16),
            "wv": np.ascontiguousarray(Wv[rows_v].T).astype(f16),
            "wo": np.ascontiguousarray(Wo[:, rows_v].T).astype(f16),
            "ropeA": ra,
            "ropeB": rb,
        })
    return in_maps


def kernel(x, rope_cos, rope_sin, Wq, Wk, Wv, Wo, _trace=False, _trace_cores=None):
    from concourse.bass_utils import run_bass_kernel_spmd

    nc = _get_nc()
    in_maps = _host_inputs(x, rope_cos, rope_sin, Wq, Wk, Wv, Wo)
    res = run_bass_kernel_spmd(nc, in_maps, list(range(NCORES)),
                               trace=_trace, trace_cores=_trace_cores)
    _CACHE["last_result"] = res

    out = np.zeros((B, S, D), np.float32)
    for core in range(NCORES):
        b = core // HPC
        out[b] += res.results[core]["y"].astype(np.float32)
    return out
